# revision 16
# baseline (speedup 1.0000x reference)
"""AtomwiseReadout distributed Trainium2 kernel.

Computes e_total = segment_sum(f @ w_e) for sorted segment ids:
  f            [N, 128] f32
  segment_ids  [N]      i32 (sorted)
  w_e          [128, 1] f32
  out          [G]      f32

Strategy (8 NeuronCores, data parallel, no collectives):
  - Host: find graph boundaries (searchsorted), split atoms across the 8
    cores at graph boundaries so each graph lives on one core. Within a
    core, graphs are grouped into windows of 128 consecutive graphs; window
    w is padded to w_sched[w] tiles of 128 atoms (max over cores, so the
    PSUM-accumulation schedule is identical on every core / SPMD).
  - Atom layout: groups of 512 consecutive atoms; partition p holds atoms
    {4p..4p+3} of its group, so every DMA reads 2 KiB contiguous per
    partition. A matmul tile k (0..3) of a group is atoms {4p + k}.
  - Device, per group (batched in chunks of 8 groups = 2 MiB):
      * ACT: cast f tile f32 -> bf16
      * DVE: one-hot sel[p, q] = (srel[p] == q), srel = graph slot in window
      * PE:  psum[slot, feat] += sel^T f  accumulated over the window
  - Window end: DVE projects psum with w (mult + reduce) -> acc[:, w].
  - Single DMA of all per-graph sums; host concatenates per-core results.
"""

import sys

if "/opt/trn_rl_repo" not in sys.path:
    sys.path.insert(0, "/opt/trn_rl_repo")

import numpy as np

P = 128
FEAT = 128
GRP = 4             # atoms per partition per group (group = GRP * P atoms)
GCHUNK = 8          # groups per chunk (8 * 512 atoms * 512B = 2 MiB)
N_CORES = 8
PAD_SLOT = 255.0    # srel value for padding atoms; never equals a slot id

_graph_cache = {}


def _build(w_sched):
    from concourse import bacc, bass, mybir, tile

    f32 = mybir.dt.float32
    bf16 = mybir.dt.bfloat16

    w_sched = list(w_sched)
    n_windows = len(w_sched)
    total_groups = sum(w_sched) // GRP
    g_pad = n_windows * P
    na_pad = total_groups * GRP * P
    n_chunks = -(-total_groups // GCHUNK)

    # tile t -> window id
    tile2win = []
    for w, nt in enumerate(w_sched):
        tile2win.extend([w] * nt)
    win_last_tile = np.cumsum(w_sched) - 1

    nc = bacc.Bacc(None)
    f_ext = nc.declare_dram_parameter("f", [na_pad, FEAT], f32, False)
    srel_ext = nc.declare_dram_parameter(
        "srel", [P, total_groups, GRP], bf16, False)
    wrep_ext = nc.declare_dram_parameter("wrep", [P, FEAT], f32, False)
    irow_ext = nc.declare_dram_parameter("irow", [P, P], bf16, False)
    out_ext = nc.declare_dram_parameter("out", [g_pad], f32, True)

    with tile.TileContext(nc) as tc:
        with tc.tile_pool(name="persist", bufs=1) as pp, \
             tc.tile_pool(name="fio", bufs=5) as fp, \
             tc.tile_pool(name="work", bufs=4) as wp, \
             tc.tile_pool(name="psum", bufs=2, space="PSUM") as psp:
            wrep_sb = pp.tile([P, FEAT], f32)
            nc.sync.dma_start(out=wrep_sb[:], in_=wrep_ext[:, :])
            irow_sb = pp.tile([P, 1, P], bf16)
            nc.sync.dma_start(out=irow_sb[:], in_=irow_ext[:, None, :])
            acc = pp.tile([P, n_windows], f32)

            psum_t = None
            last_set = set(int(x) for x in win_last_tile)
            # chunk plan: full chunks, with a small final chunk so the
            # post-DMA tail (cast/select/matmul of the last chunk) is short
            plan = []
            cs0 = 0
            while cs0 < total_groups:
                gct0 = min(GCHUNK, total_groups - cs0)
                plan.append((cs0, gct0))
                cs0 += gct0
            if len(plan) > 1 and plan[-1][1] > 2:
                cs0, gct0 = plan.pop()
                plan.append((cs0, gct0 - 2))
                plan.append((cs0 + gct0 - 2, 2))
            for cs, gct in plan:
                fch = fp.tile([P, GCHUNK, GRP, FEAT], f32, tag="fch")
                nc.sync.dma_start(
                    out=fch[:, :gct, :, :],
                    in_=bass.AP(
                        f_ext, cs * GRP * P * FEAT,
                        [(GRP * FEAT, P), (GRP * P * FEAT, gct),
                         (FEAT, GRP), (1, FEAT)],
                    ),
                )
                srel_sb = fp.tile([P, GCHUNK, GRP], bf16, tag="srel")
                nc.sync.dma_start(
                    out=srel_sb[:, :gct, :], in_=srel_ext[:, cs:cs + gct, :]
                )
                # cast f -> bf16 on the otherwise-idle Scalar/ACT engine
                fbf = wp.tile([P, GCHUNK, GRP, FEAT], bf16, tag="fbf")
                nc.scalar.copy(out=fbf[:, :gct, :, :], in_=fch[:, :gct, :, :])
                sel = wp.tile([P, GCHUNK, GRP, P], bf16, tag="sel")
                nc.vector.tensor_tensor(
                    out=bass.AP(
                        sel[:].tensor, sel[:].offset,
                        [sel[:].ap[0], (P, gct * GRP), (1, P)],
                    ),
                    in0=irow_sb[:].to_broadcast([P, gct * GRP, P]),
                    in1=bass.AP(
                        srel_sb[:].tensor, srel_sb[:].offset,
                        [srel_sb[:].ap[0], (1, gct * GRP), (0, P)],
                    ),
                    op=mybir.AluOpType.is_equal,
                )
                for j in range(gct):
                    for k in range(GRP):
                        t = (cs + j) * GRP + k
                        w = tile2win[t]
                        start = (t == 0) or (tile2win[t - 1] != w)
                        stop = t in last_set
                        if start:
                            psum_t = psp.tile([P, FEAT], f32, tag="ps")
                        # psum[slot, feat] += sum_a sel[a, slot] * f[a, feat]
                        nc.tensor.matmul(
                            out=psum_t[:],
                            lhsT=sel[:, j, k, :],
                            rhs=fbf[:, j, k, :],
                            start=start,
                            stop=stop,
                        )
                        if stop:
                            # project window's per-slot feature sums with w
                            scr = wp.tile([P, FEAT], f32, tag="scr")
                            nc.vector.tensor_tensor(
                                out=scr[:],
                                in0=psum_t[:],
                                in1=wrep_sb[:],
                                op=mybir.AluOpType.mult,
                            )
                            nc.vector.tensor_reduce(
                                out=acc[:, w:w + 1],
                                in_=scr[:, None, :],
                                axis=mybir.AxisListType.X,
                                op=mybir.AluOpType.add,
                            )
            nc.sync.dma_start(
                out=bass.AP(out_ext, 0, [(1, P), (P, n_windows)]), in_=acc[:]
            )
    if not nc.is_finalized():
        nc.finalize()
    return nc


def _prepare(f, segment_ids, n_graphs, w_e):
    f = np.ascontiguousarray(np.asarray(f, dtype=np.float32))
    seg = np.asarray(segment_ids, dtype=np.int64)
    w = np.asarray(w_e, dtype=np.float32).reshape(FEAT)
    G = int(n_graphs)
    N = f.shape[0]

    # graph g owns atoms [b[g], b[g+1])
    b = np.searchsorted(seg, np.arange(G + 1), side="left")
    # split graphs across cores at ~equal atom counts
    gedges = [0]
    for k in range(1, N_CORES):
        gedges.append(int(np.searchsorted(b, (N * k) // N_CORES)))
    gedges.append(G)
    gedges = np.maximum.accumulate(np.array(gedges, dtype=np.int64))

    ng = np.diff(gedges)
    n_windows = max(-(-int(ng.max()) // P), 1)

    # per-window atom ranges; schedule = per-window max tile count over
    # cores, rounded up to whole groups
    atoms_per_group = GRP * P
    win_ranges = []  # [core][window] = (a_lo, a_hi, g0)
    w_sched = [1] * n_windows
    for c in range(N_CORES):
        gs, ge = int(gedges[c]), int(gedges[c + 1])
        rows = []
        for wdx in range(n_windows):
            g0 = gs + wdx * P
            g1 = min(g0 + P, ge)
            if g0 >= ge:
                rows.append((0, 0, g0))
                continue
            alo, ahi = int(b[g0]), int(b[g1])
            rows.append((alo, ahi, g0))
            w_sched[wdx] = max(
                w_sched[wdx], -(-(ahi - alo) // atoms_per_group))
        win_ranges.append(rows)
    w_sched = [wg * GRP for wg in w_sched]  # group counts -> tile counts
    win_off = np.concatenate([[0], np.cumsum(w_sched)]) * P  # atom offsets

    total_tiles = sum(w_sched)
    total_groups = total_tiles // GRP
    na_pad = total_tiles * P

    import ml_dtypes

    bf16 = ml_dtypes.bfloat16
    wrep = np.ascontiguousarray(np.broadcast_to(w[None, :], (P, FEAT)), np.float32)
    irow = np.ascontiguousarray(
        np.broadcast_to(np.arange(P, dtype=np.float32)[None, :], (P, P))
    ).astype(bf16)

    in_maps = []
    for c in range(N_CORES):
        f_pad = np.zeros((na_pad, FEAT), np.float32)
        srel = np.full(na_pad, PAD_SLOT, np.float32)
        for wdx, (alo, ahi, g0) in enumerate(win_ranges[c]):
            n = ahi - alo
            if n == 0:
                continue
            dst = int(win_off[wdx])
            f_pad[dst:dst + n] = f[alo:ahi]
            srel[dst:dst + n] = (seg[alo:ahi] - g0).astype(np.float32)
        # srel[group*512 + 4p + k] -> srel_t[p, group, k]
        srel_t = np.ascontiguousarray(
            srel.reshape(total_groups, P, GRP).transpose(1, 0, 2)
        ).astype(bf16)
        in_maps.append({
            "f": f_pad,
            "srel": srel_t,
            "wrep": wrep,
            "irow": irow,
        })
    return in_maps, gedges, tuple(w_sched)


def kernel(f, segment_ids, n_graphs, w_e, _trace=False):
    from concourse.bass_utils import run_bass_kernel_spmd

    in_maps, gedges, w_sched = _prepare(f, segment_ids, n_graphs, w_e)

    if w_sched not in _graph_cache:
        _graph_cache[w_sched] = _build(w_sched)
    nc = _graph_cache[w_sched]

    res = run_bass_kernel_spmd(
        nc, in_maps, core_ids=list(range(N_CORES)), trace=_trace
    )
    G = int(n_graphs)
    out = np.empty(G, np.float32)
    for c in range(N_CORES):
        gs, ge = int(gedges[c]), int(gedges[c + 1])
        out[gs:ge] = np.asarray(res.results[c]["out"]).ravel()[: ge - gs]
    if _trace:
        return out, res
    return out


# revision 18
# speedup vs baseline: 1.0819x; 1.0819x over previous
"""AtomwiseReadout distributed Trainium2 kernel.

Computes e_total = segment_sum(f @ w_e) for sorted segment ids:
  f            [N, 128] f32
  segment_ids  [N]      i32 (sorted)
  w_e          [128, 1] f32
  out          [G]      f32

Strategy (8 NeuronCores, data parallel, no collectives):
  - Host: find graph boundaries (searchsorted), split atoms across the 8
    cores at graph boundaries so each graph lives on one core. Within a
    core, graphs are grouped into windows of 128 consecutive graphs; window
    w is padded to w_sched[w] tiles of 128 atoms (max over cores, so the
    PSUM-accumulation schedule is identical on every core / SPMD).
  - Atom layout: groups of 512 consecutive atoms; partition p holds atoms
    {4p..4p+3} of its group, so every DMA reads 2 KiB contiguous per
    partition. A matmul tile k (0..3) of a group is atoms {4p + k}.
  - Device, per group (batched in chunks of 8 groups = 2 MiB):
      * ACT: cast f tile f32 -> bf16
      * DVE: one-hot sel[p, q] = (srel[p] == q), srel = graph slot in window
      * PE:  psum[slot, feat] += sel^T f  accumulated over the window
  - Window end: DVE projects psum with w (mult + reduce) -> acc[:, w].
  - Single DMA of all per-graph sums; host concatenates per-core results.
"""

import sys

if "/opt/trn_rl_repo" not in sys.path:
    sys.path.insert(0, "/opt/trn_rl_repo")

import numpy as np

P = 128
FEAT = 128
GRP = 4             # atoms per partition per group (group = GRP * P atoms)
GCHUNK = 8          # groups per chunk (8 * 512 atoms * 512B = 2 MiB)
N_CORES = 8
PAD_SLOT = 255.0    # srel value for padding atoms; never equals a slot id

_graph_cache = {}


def _build(w_sched):
    from concourse import bacc, bass, mybir, tile

    f32 = mybir.dt.float32
    bf16 = mybir.dt.bfloat16

    w_sched = list(w_sched)
    n_windows = len(w_sched)
    total_groups = sum(w_sched) // GRP
    g_pad = n_windows * P
    na_pad = total_groups * GRP * P
    n_chunks = -(-total_groups // GCHUNK)

    # tile t -> window id
    tile2win = []
    for w, nt in enumerate(w_sched):
        tile2win.extend([w] * nt)
    win_last_tile = np.cumsum(w_sched) - 1

    nc = bacc.Bacc(None)
    f_ext = nc.declare_dram_parameter("f", [na_pad, FEAT], f32, False)
    srel_ext = nc.declare_dram_parameter(
        "srel", [P, total_groups, GRP], bf16, False)
    wrep_ext = nc.declare_dram_parameter("wrep", [P, FEAT], f32, False)
    irow_ext = nc.declare_dram_parameter("irow", [P, P], bf16, False)
    out_ext = nc.declare_dram_parameter("out", [g_pad], f32, True)

    with tile.TileContext(nc) as tc:
        with tc.tile_pool(name="persist", bufs=1) as pp, \
             tc.tile_pool(name="fio", bufs=4) as fp, \
             tc.tile_pool(name="work", bufs=3) as wp, \
             tc.tile_pool(name="psum", bufs=2, space="PSUM") as psp:
            wrep_sb = pp.tile([P, FEAT], f32)
            nc.sync.dma_start(out=wrep_sb[:], in_=wrep_ext[:, :])
            irow_sb = pp.tile([P, 1, P], bf16)
            nc.sync.dma_start(out=irow_sb[:], in_=irow_ext[:, None, :])
            acc = pp.tile([P, n_windows], f32)

            psum_t = None
            last_set = set(int(x) for x in win_last_tile)
            # chunk plan: full chunks, with a small final chunk so the
            # post-DMA tail (cast/select/matmul of the last chunk) is short
            plan = []
            cs0 = 0
            while cs0 < total_groups:
                gct0 = min(GCHUNK, total_groups - cs0)
                plan.append((cs0, gct0))
                cs0 += gct0
            for cs, gct in plan:
                fch = fp.tile([P, GCHUNK, GRP, FEAT], f32, tag="fch")
                nc.sync.dma_start(
                    out=fch[:, :gct, :, :],
                    in_=bass.AP(
                        f_ext, cs * GRP * P * FEAT,
                        [(GRP * FEAT, P), (GRP * P * FEAT, gct),
                         (FEAT, GRP), (1, FEAT)],
                    ),
                )
                srel_sb = fp.tile([P, GCHUNK, GRP], bf16, tag="srel")
                nc.sync.dma_start(
                    out=srel_sb[:, :gct, :], in_=srel_ext[:, cs:cs + gct, :]
                )
                # cast f -> bf16 on the otherwise-idle Scalar/ACT engine
                fbf = wp.tile([P, GCHUNK, GRP, FEAT], bf16, tag="fbf")
                nc.scalar.copy(out=fbf[:, :gct, :, :], in_=fch[:, :gct, :, :])
                sel = wp.tile([P, GCHUNK, GRP, P], bf16, tag="sel")
                nc.vector.tensor_tensor(
                    out=bass.AP(
                        sel[:].tensor, sel[:].offset,
                        [sel[:].ap[0], (P, gct * GRP), (1, P)],
                    ),
                    in0=irow_sb[:].to_broadcast([P, gct * GRP, P]),
                    in1=bass.AP(
                        srel_sb[:].tensor, srel_sb[:].offset,
                        [srel_sb[:].ap[0], (1, gct * GRP), (0, P)],
                    ),
                    op=mybir.AluOpType.is_equal,
                )
                for j in range(gct):
                    for k in range(GRP):
                        t = (cs + j) * GRP + k
                        w = tile2win[t]
                        start = (t == 0) or (tile2win[t - 1] != w)
                        stop = t in last_set
                        if start:
                            psum_t = psp.tile([P, FEAT], f32, tag="ps")
                        # psum[slot, feat] += sum_a sel[a, slot] * f[a, feat]
                        nc.tensor.matmul(
                            out=psum_t[:],
                            lhsT=sel[:, j, k, :],
                            rhs=fbf[:, j, k, :],
                            start=start,
                            stop=stop,
                        )
                        if stop:
                            # project window's per-slot feature sums with w
                            scr = wp.tile([P, FEAT], f32, tag="scr")
                            nc.vector.tensor_tensor(
                                out=scr[:],
                                in0=psum_t[:],
                                in1=wrep_sb[:],
                                op=mybir.AluOpType.mult,
                            )
                            nc.vector.tensor_reduce(
                                out=acc[:, w:w + 1],
                                in_=scr[:, None, :],
                                axis=mybir.AxisListType.X,
                                op=mybir.AluOpType.add,
                            )
            nc.sync.dma_start(
                out=bass.AP(out_ext, 0, [(1, P), (P, n_windows)]), in_=acc[:]
            )
    if not nc.is_finalized():
        nc.finalize()
    return nc


def _prepare(f, segment_ids, n_graphs, w_e):
    f = np.ascontiguousarray(np.asarray(f, dtype=np.float32))
    seg = np.asarray(segment_ids, dtype=np.int64)
    w = np.asarray(w_e, dtype=np.float32).reshape(FEAT)
    G = int(n_graphs)
    N = f.shape[0]

    # graph g owns atoms [b[g], b[g+1])
    b = np.searchsorted(seg, np.arange(G + 1), side="left")
    # split graphs across cores at ~equal atom counts
    gedges = [0]
    for k in range(1, N_CORES):
        gedges.append(int(np.searchsorted(b, (N * k) // N_CORES)))
    gedges.append(G)
    gedges = np.maximum.accumulate(np.array(gedges, dtype=np.int64))

    ng = np.diff(gedges)
    n_windows = max(-(-int(ng.max()) // P), 1)

    # per-window atom ranges; schedule = per-window max tile count over
    # cores, rounded up to whole groups
    atoms_per_group = GRP * P
    win_ranges = []  # [core][window] = (a_lo, a_hi, g0)
    w_sched = [1] * n_windows
    for c in range(N_CORES):
        gs, ge = int(gedges[c]), int(gedges[c + 1])
        rows = []
        for wdx in range(n_windows):
            g0 = gs + wdx * P
            g1 = min(g0 + P, ge)
            if g0 >= ge:
                rows.append((0, 0, g0))
                continue
            alo, ahi = int(b[g0]), int(b[g1])
            rows.append((alo, ahi, g0))
            w_sched[wdx] = max(
                w_sched[wdx], -(-(ahi - alo) // atoms_per_group))
        win_ranges.append(rows)
    w_sched = [wg * GRP for wg in w_sched]  # group counts -> tile counts
    win_off = np.concatenate([[0], np.cumsum(w_sched)]) * P  # atom offsets

    total_tiles = sum(w_sched)
    total_groups = total_tiles // GRP
    na_pad = total_tiles * P

    import ml_dtypes

    bf16 = ml_dtypes.bfloat16
    wrep = np.ascontiguousarray(np.broadcast_to(w[None, :], (P, FEAT)), np.float32)
    irow = np.ascontiguousarray(
        np.broadcast_to(np.arange(P, dtype=np.float32)[None, :], (P, P))
    ).astype(bf16)

    in_maps = []
    for c in range(N_CORES):
        f_pad = np.zeros((na_pad, FEAT), np.float32)
        srel = np.full(na_pad, PAD_SLOT, np.float32)
        for wdx, (alo, ahi, g0) in enumerate(win_ranges[c]):
            n = ahi - alo
            if n == 0:
                continue
            dst = int(win_off[wdx])
            f_pad[dst:dst + n] = f[alo:ahi]
            srel[dst:dst + n] = (seg[alo:ahi] - g0).astype(np.float32)
        # srel[group*512 + 4p + k] -> srel_t[p, group, k]
        srel_t = np.ascontiguousarray(
            srel.reshape(total_groups, P, GRP).transpose(1, 0, 2)
        ).astype(bf16)
        in_maps.append({
            "f": f_pad,
            "srel": srel_t,
            "wrep": wrep,
            "irow": irow,
        })
    return in_maps, gedges, tuple(w_sched)


def kernel(f, segment_ids, n_graphs, w_e, _trace=False):
    from concourse.bass_utils import run_bass_kernel_spmd

    in_maps, gedges, w_sched = _prepare(f, segment_ids, n_graphs, w_e)

    if w_sched not in _graph_cache:
        _graph_cache[w_sched] = _build(w_sched)
    nc = _graph_cache[w_sched]

    res = run_bass_kernel_spmd(
        nc, in_maps, core_ids=list(range(N_CORES)), trace=_trace
    )
    G = int(n_graphs)
    out = np.empty(G, np.float32)
    for c in range(N_CORES):
        gs, ge = int(gedges[c]), int(gedges[c + 1])
        out[gs:ge] = np.asarray(res.results[c]["out"]).ravel()[: ge - gs]
    if _trace:
        return out, res
    return out
